# revision 8
# baseline (speedup 1.0000x reference)
"""Trainium2 Bass kernel for the GRU decoder (nn_Decoder_13168369730058).

Math (from the reference):
  h0 = encoder_outputs[0, :, -1, :]                       # (128, 512)
  step 1:   h1 = gru_cell(x=0, h0)
  step t>1: h_t = gru_cell(h_{t-1}, h_{t-1})   (carry is (h_new, h_new))

Because x == h from step 2 on, the two GRU matmuls fuse into one:
  g  = h @ Wc.T          Wc = [Wih_r+Whh_r; Wih_z+Whh_z; Whh_n; Wih_n]  (2048, 512)
  r  = sigmoid(g_r + b_r)        b_r = b_ih_r + b_hh_r
  z  = sigmoid(g_z + b_z)
  n  = tanh(g_in + b_in + r * (g_hn + b_hn))     b_in = b_ih_n, b_hn = b_hh_n
  h' = (1 - z) * n + z * h
Step 1 is the same recurrence with Wc -> W_hh and no in-matmul (g_in = 0).

Distribution: data-parallel over batch, 16 rows per core on 8 cores, weights
replicated; the out_len recurrence is local to each core.

v2 design (from trace analysis of v1: the 64-matmul weight stream runs at the
~27ns/pair PE issue floor = 1.7us, but the serial gate-math tail added 2.2us
of PE idle per step):
  - k-OUTER matmul ordering: phase k uses only h16 chunk k as the moving
    operand, so the next step's stream starts as soon as tail chunk 0 is done.
  - phase k3 is chunk-major (r0,z0,r1,z1,hn0,in0,...) so gate tiles complete
    progressively and the tail starts ~300ns before the stream ends.
  - tail is chunked by k-tile and interleaved across ACT/DVE so h16 chunk 0
    closes the recurrence ~700ns after the stream, with later chunks hidden
    under the next stream's phases.
  - b_hn/b_in biases folded into scalar_tensor_tensor ops; b_r/b_z via one
    cheap bias-seed matmul so one ACT covers r and z sigmoid per half.
  - h carried purely in fp16 (numpy sim: rel err 5.5e-4 vs 2e-2 budget);
    output DMA'd as fp16 and widened to fp32 on the host.
"""

import os
import numpy as np

import concourse.bacc as bacc
import concourse.mybir as mybir
import concourse.tile as tile
from concourse.bass_utils import run_bass_kernel_spmd

H = 512
BATCH = 128
N_CORES = int(os.environ.get("GRU_N_CORES", "8"))
T_STEPS = int(os.environ.get("GRU_T_STEPS", "1024"))
B_LOC = BATCH // N_CORES  # local batch per core (16)
KT = H // 128             # 4 k-tiles

F32 = mybir.dt.float32
F16 = mybir.dt.float16

# full-step stream tile order within each k phase (gate, chunk)
TILE_ORDER = [
    ("r", 0), ("z", 0), ("r", 1), ("z", 1),
    ("hn", 0), ("in", 0), ("hn", 1), ("in", 1),
    ("r", 2), ("z", 2), ("r", 3), ("z", 3),
    ("hn", 2), ("in", 2), ("hn", 3), ("in", 3),
]
# step-1 order (no in gate)
TILE_ORDER1 = [
    ("r", 0), ("z", 0), ("r", 1), ("z", 1), ("hn", 0), ("hn", 1),
    ("r", 2), ("z", 2), ("r", 3), ("z", 3), ("hn", 2), ("hn", 3),
]


def _build(T: int, b: int):
    nc = bacc.Bacc()

    wc_d = nc.dram_tensor("wc", [128, 64 * 128], F16, kind="ExternalInput")
    w1_d = nc.dram_tensor("w1", [128, 48 * 128], F16, kind="ExternalInput")
    # bias stationaries: rz rows [br0,br1,bz0,bz1,br2,br3,bz2,bz3]
    bst_d = nc.dram_tensor("bst", [8, 128], F16, kind="ExternalInput")
    bhn_d = nc.dram_tensor("bhnst", [4, 128], F16, kind="ExternalInput")
    bin_d = nc.dram_tensor("binst", [4, 128], F16, kind="ExternalInput")
    ones8_d = nc.dram_tensor("ones8", [8, 8 * b], F16, kind="ExternalInput")
    ones4_d = nc.dram_tensor("ones4", [4, 4 * b], F16, kind="ExternalInput")
    h0_d = nc.dram_tensor("h0t", [128, 4 * b], F16, kind="ExternalInput")
    out_d = nc.dram_tensor("outT", [T, 128, 4 * b], F16, kind="ExternalOutput")

    sig = mybir.ActivationFunctionType.Sigmoid
    tanh = mybir.ActivationFunctionType.Tanh
    ADD = mybir.AluOpType.add
    MULT = mybir.AluOpType.mult

    with tile.TileContext(nc) as tc:
        with (
            tc.tile_pool(name="singles", bufs=1) as singles,
            tc.tile_pool(name="state", bufs=2) as state,
            tc.tile_pool(name="work", bufs=2) as work,
            tc.tile_pool(name="psum", bufs=2, space="PSUM") as psum,
        ):
            w_sb = singles.tile([128, 64 * 128], F16)
            nc.sync.dma_start(w_sb[:], wc_d[:])
            w1_sb = singles.tile([128, 48 * 128], F16)
            nc.sync.dma_start(w1_sb[:], w1_d[:])
            bst_sb = singles.tile([8, 128], F16)
            nc.sync.dma_start(bst_sb[:], bst_d[:])
            bhn_st = singles.tile([4, 128], F16)
            nc.sync.dma_start(bhn_st[:], bhn_d[:])
            bin_st = singles.tile([4, 128], F16)
            nc.sync.dma_start(bin_st[:], bin_d[:])
            ones8_sb = singles.tile([8, 8 * b], F16)
            nc.sync.dma_start(ones8_sb[:], ones8_d[:])
            ones4_sb = singles.tile([4, 4 * b], F16)
            nc.sync.dma_start(ones4_sb[:], ones4_d[:])

            h16 = state.tile([128, 4 * b], F16, tag="h16")
            nc.sync.dma_start(h16[:], h0_d[:])

            # Warm-up: have each engine observe the init DMA queues so loop
            # instructions carry at most one embedded cross-engine wait.
            warm_ps = psum.tile([128, 8], F32, tag="warm", bufs=1)
            nc.tensor.matmul(warm_ps[:, 0:8], w_sb[:, 0:128], w_sb[:, 0:8],
                             start=True, stop=True)
            nc.tensor.matmul(warm_ps[:, 0:8], w1_sb[:, 0:128], w1_sb[:, 0:8],
                             start=True, stop=True)
            nc.tensor.matmul(warm_ps[:, 0:1], bst_sb[:, 0:128],
                             ones8_sb[:, 0:1], start=True, stop=True)
            nc.tensor.matmul(warm_ps[:, 0:1], bhn_st[:, 0:128],
                             ones4_sb[:, 0:1], start=True, stop=True)
            nc.tensor.matmul(warm_ps[:, 0:1], bin_st[:, 0:128],
                             ones4_sb[:, 0:1], start=True, stop=True)

            for t in range(T):
                first = t == 0
                order = TILE_ORDER1 if first else TILE_ORDER
                ntiles = len(order)
                w = w1_sb if first else w_sb

                rz_ps = psum.tile([128, 8 * b], F32, tag="rz")
                hn_ps = psum.tile([128, 4 * b], F32, tag="hn")
                in_ps = psum.tile([128, 4 * b], F32, tag="in")
                if first:
                    # in gate is bias-only at t=0 (x=0): seed closes its group
                    nc.tensor.matmul(in_ps[:], bin_st[:], ones4_sb[:],
                                     start=True, stop=True,
                                     skip_group_check=True)

                # bias seeds: every start=True for a PSUM bank must precede
                # all accumulating MMs of that bank (first_mm clears the whole
                # bank's has_written bits, not just the addressed region)
                nc.tensor.matmul(rz_ps[:], bst_sb[:], ones8_sb[:],
                                 start=True, stop=False, skip_group_check=True)
                nc.tensor.matmul(hn_ps[:], bhn_st[:], ones4_sb[:],
                                 start=True, stop=False, skip_group_check=True)
                if not first:
                    nc.tensor.matmul(in_ps[:], bin_st[:], ones4_sb[:],
                                     start=True, stop=False,
                                     skip_group_check=True)

                for k in range(KT):
                    mv = h16[:, k * b:(k + 1) * b]
                    for ti, (g, c) in enumerate(order):
                        blk = (k * ntiles + ti) * 128
                        if g == "r":
                            ps, off = rz_ps, (c % 2) * b + (c // 2) * 4 * b
                        elif g == "z":
                            ps, off = rz_ps, 2 * b + (c % 2) * b + (c // 2) * 4 * b
                        elif g == "hn":
                            ps, off = hn_ps, c * b
                        else:
                            ps, off = in_ps, c * b
                        nc.tensor.matmul(
                            ps[:, off:off + b],
                            w[:, blk:blk + 128],
                            mv,
                            start=False,
                            stop=(k == KT - 1),
                            skip_group_check=True,
                        )

                rzs = work.tile([128, 8 * b], F32, tag="rzs")
                rhn = work.tile([128, 4 * b], F32, tag="rhn")
                pre = work.tile([128, 4 * b], F32, tag="pre")
                n_t = work.tile([128, 4 * b], F32, tag="n")
                omz = work.tile([128, 4 * b], F32, tag="omz")
                zh = work.tile([128, 4 * b], F32, tag="zh")
                on = work.tile([128, 4 * b], F32, tag="on")
                h16_new = state.tile([128, 4 * b], F16, tag="h16")

                # rz layout: [r0,r1,z0,z1 | r2,r3,z2,z3]
                def rcol(c):
                    return (c % 2) * b + (c // 2) * 4 * b

                def zcol(c):
                    return 2 * b + (c % 2) * b + (c // 2) * 4 * b

                def chunk_pre(c):
                    cb = slice(c * b, (c + 1) * b)
                    nc.vector.tensor_mul(rhn[:, cb], hn_ps[:, cb],
                                         rzs[:, rcol(c):rcol(c) + b])
                    nc.vector.tensor_add(pre[:, cb], in_ps[:, cb], rhn[:, cb])
                    nc.scalar.activation(n_t[:, cb], pre[:, cb], tanh)

                def chunk_post(c):
                    cb = slice(c * b, (c + 1) * b)
                    nc.vector.tensor_mul(on[:, cb], omz[:, cb], n_t[:, cb])
                    nc.vector.tensor_add(h16_new[:, cb], on[:, cb], zh[:, cb])

                def half_oz(hf):
                    hb = slice(2 * hf * b, 2 * (hf + 1) * b)
                    zb = slice(zcol(2 * hf), zcol(2 * hf) + 2 * b)
                    nc.vector.tensor_scalar(omz[:, hb], rzs[:, zb], -1.0, 1.0,
                                            MULT, ADD)
                    nc.vector.tensor_mul(zh[:, hb], rzs[:, zb], h16[:, hb])

                nc.scalar.activation(rzs[:, 0:4 * b], rz_ps[:, 0:4 * b], sig)
                chunk_pre(0)
                chunk_pre(1)
                half_oz(0)
                nc.scalar.activation(rzs[:, 4 * b:8 * b], rz_ps[:, 4 * b:8 * b],
                                     sig)
                chunk_post(0)
                chunk_post(1)
                chunk_pre(2)
                chunk_pre(3)
                half_oz(1)
                chunk_post(2)
                chunk_post(3)

                nc.sync.dma_start(out_d[t], h16_new[:])
                h16 = h16_new

    if not nc.is_finalized():
        nc.finalize()
    return nc


def _prep_host(encoder_outputs, W_ih, W_hh, b_ih, b_hh, T, n_cores, b):
    """Shard + lay out host inputs; returns per-core in_maps."""
    W_ih = np.asarray(W_ih, dtype=np.float32)
    W_hh = np.asarray(W_hh, dtype=np.float32)
    b_ih = np.asarray(b_ih, dtype=np.float32)
    b_hh = np.asarray(b_hh, dtype=np.float32)
    enc = np.asarray(encoder_outputs, dtype=np.float32)

    gates = {
        "r": W_ih[:H] + W_hh[:H],
        "z": W_ih[H:2 * H] + W_hh[H:2 * H],
        "hn": W_hh[2 * H:],
        "in": W_ih[2 * H:],
    }
    gates1 = {
        "r": W_hh[:H],
        "z": W_hh[H:2 * H],
        "hn": W_hh[2 * H:],
    }
    b_r = b_ih[:H] + b_hh[:H]
    b_z = b_ih[H:2 * H] + b_hh[H:2 * H]
    b_hn = b_hh[2 * H:]
    b_in = b_ih[2 * H:]

    def layout(gmap, order):
        cols = []
        for k in range(KT):
            for g, c in order:
                Wg = gmap[g]
                blkT = Wg[128 * c:128 * (c + 1), 128 * k:128 * (k + 1)].T
                cols.append(np.ascontiguousarray(blkT))
        return np.concatenate(cols, axis=1).astype(np.float16)

    wc_host = layout(gates, TILE_ORDER)     # (128, 64*128)
    w1_host = layout(gates1, TILE_ORDER1)   # (128, 48*128)

    br4 = b_r.reshape(4, 128)
    bz4 = b_z.reshape(4, 128)
    # rz rows [br0,br1,bz0,bz1, br2,br3,bz2,bz3] matching the psum layout
    bst = np.stack([br4[0], br4[1], bz4[0], bz4[1],
                    br4[2], br4[3], bz4[2], bz4[3]],
                   axis=0).astype(np.float16)  # (8, 128)
    bhnst = b_hn.reshape(4, 128).astype(np.float16)
    binst = b_in.reshape(4, 128).astype(np.float16)
    ones8 = np.kron(np.eye(8, dtype=np.float16), np.ones((1, b), np.float16))
    ones4 = np.kron(np.eye(4, dtype=np.float16), np.ones((1, b), np.float16))

    h0 = enc[0, :, -1, :]  # (128, 512)
    in_maps = []
    for c in range(n_cores):
        h0c = h0[c * b:(c + 1) * b]  # (b, 512)
        h0t = np.ascontiguousarray(
            h0c.reshape(b, KT, 128).transpose(2, 1, 0).reshape(128, KT * b)
        ).astype(np.float16)
        in_maps.append({
            "wc": wc_host, "w1": w1_host, "bst": bst, "bhnst": bhnst,
            "binst": binst, "ones8": ones8, "ones4": ones4, "h0t": h0t,
        })
    return in_maps


def _gather(results, T, n_cores, b):
    out = np.empty((T, BATCH, H), dtype=np.float32)
    for c in range(n_cores):
        oc = results[c]["outT"].astype(np.float32)  # (T, 128, KT*b)
        out[:, c * b:(c + 1) * b, :] = (
            oc.reshape(T, 128, KT, b).transpose(0, 3, 2, 1).reshape(T, b, H)
        )
    return out


_CACHE = {}


def kernel(encoder_outputs, W_ih, W_hh, b_ih, b_hh, out_len):
    T = int(out_len)
    assert T == T_STEPS, f"built for T={T_STEPS}, got {T}"
    key = (T, N_CORES)
    if key not in _CACHE:
        _CACHE[key] = _build(T, B_LOC)
    nc = _CACHE[key]

    in_maps = _prep_host(encoder_outputs, W_ih, W_hh, b_ih, b_hh,
                         T, N_CORES, B_LOC)
    res = run_bass_kernel_spmd(nc, in_maps, core_ids=list(range(N_CORES)))
    global LAST_RESULTS
    LAST_RESULTS = res
    out = _gather(res.results, T, N_CORES, B_LOC)
    return out.reshape(T * BATCH, 1, H)


LAST_RESULTS = None


# revision 9
# speedup vs baseline: 1.0010x; 1.0010x over previous
"""Trainium2 Bass kernel for the GRU decoder (nn_Decoder_13168369730058).

Math (from the reference):
  h0 = encoder_outputs[0, :, -1, :]                       # (128, 512)
  step 1:   h1 = gru_cell(x=0, h0)
  step t>1: h_t = gru_cell(h_{t-1}, h_{t-1})   (carry is (h_new, h_new))

Because x == h from step 2 on, the two GRU matmuls fuse into one:
  g  = h @ Wc.T          Wc = [Wih_r+Whh_r; Wih_z+Whh_z; Whh_n; Wih_n]  (2048, 512)
  r  = sigmoid(g_r + b_r)        b_r = b_ih_r + b_hh_r
  z  = sigmoid(g_z + b_z)
  n  = tanh(g_in + b_in + r * (g_hn + b_hn))     b_in = b_ih_n, b_hn = b_hh_n
  h' = (1 - z) * n + z * h
Step 1 is the same recurrence with Wc -> W_hh and no in-matmul (g_in = 0).

Distribution: data-parallel over batch, 16 rows per core on 8 cores, weights
replicated; the out_len recurrence is local to each core.

v2 design (from trace analysis of v1: the 64-matmul weight stream runs at the
~27ns/pair PE issue floor = 1.7us, but the serial gate-math tail added 2.2us
of PE idle per step):
  - k-OUTER matmul ordering: phase k uses only h16 chunk k as the moving
    operand, so the next step's stream starts as soon as tail chunk 0 is done.
  - phase k3 is chunk-major (r0,z0,r1,z1,hn0,in0,...) so gate tiles complete
    progressively and the tail starts ~300ns before the stream ends.
  - tail is chunked by k-tile and interleaved across ACT/DVE so h16 chunk 0
    closes the recurrence ~700ns after the stream, with later chunks hidden
    under the next stream's phases.
  - b_hn/b_in biases folded into scalar_tensor_tensor ops; b_r/b_z via one
    cheap bias-seed matmul so one ACT covers r and z sigmoid per half.
  - h carried purely in fp16 (numpy sim: rel err 5.5e-4 vs 2e-2 budget);
    output DMA'd as fp16 and widened to fp32 on the host.
"""

import os
import numpy as np

import concourse.bacc as bacc
import concourse.mybir as mybir
import concourse.tile as tile
from concourse.bass_utils import run_bass_kernel_spmd

H = 512
BATCH = 128
N_CORES = int(os.environ.get("GRU_N_CORES", "8"))
T_STEPS = int(os.environ.get("GRU_T_STEPS", "1024"))
B_LOC = BATCH // N_CORES  # local batch per core (16)
KT = H // 128             # 4 k-tiles

F32 = mybir.dt.float32
F16 = mybir.dt.float16

# full-step stream tile order within each k phase: chunk-major so the tail
# chunks' inputs complete progressively through phase k3
TILE_ORDER = [
    ("r", 0), ("z", 0), ("hn", 0), ("in", 0),
    ("r", 1), ("z", 1), ("hn", 1), ("in", 1),
    ("r", 2), ("z", 2), ("hn", 2), ("in", 2),
    ("r", 3), ("z", 3), ("hn", 3), ("in", 3),
]
# step-1 order (no in gate)
TILE_ORDER1 = [
    ("r", 0), ("z", 0), ("hn", 0), ("r", 1), ("z", 1), ("hn", 1),
    ("r", 2), ("z", 2), ("hn", 2), ("r", 3), ("z", 3), ("hn", 3),
]


def _build(T: int, b: int):
    nc = bacc.Bacc()

    wc_d = nc.dram_tensor("wc", [128, 64 * 128], F16, kind="ExternalInput")
    w1_d = nc.dram_tensor("w1", [128, 48 * 128], F16, kind="ExternalInput")
    # bias stationaries: rz rows [br0,br1,bz0,bz1,br2,br3,bz2,bz3]
    bst_d = nc.dram_tensor("bst", [8, 128], F16, kind="ExternalInput")
    # hn+in bias stationary: rows [bhn0..3, bin0..3]
    bni_d = nc.dram_tensor("bnist", [8, 128], F16, kind="ExternalInput")
    ones8_d = nc.dram_tensor("ones8", [8, 8 * b], F16, kind="ExternalInput")
    ones4_d = nc.dram_tensor("ones4", [4, 4 * b], F16, kind="ExternalInput")
    h0_d = nc.dram_tensor("h0t", [128, 4 * b], F16, kind="ExternalInput")
    out_d = nc.dram_tensor("outT", [T, 128, 4 * b], F16, kind="ExternalOutput")

    sig = mybir.ActivationFunctionType.Sigmoid
    tanh = mybir.ActivationFunctionType.Tanh
    ADD = mybir.AluOpType.add
    MULT = mybir.AluOpType.mult

    with tile.TileContext(nc) as tc:
        with (
            tc.tile_pool(name="singles", bufs=1) as singles,
            tc.tile_pool(name="state", bufs=2) as state,
            tc.tile_pool(name="work", bufs=2) as work,
            tc.tile_pool(name="ps_rz", bufs=3, space="PSUM") as ps_rz_pool,
            tc.tile_pool(name="ps_ni", bufs=3, space="PSUM") as ps_ni_pool,
            tc.tile_pool(name="psum", bufs=1, space="PSUM") as psum,
        ):
            w_sb = singles.tile([128, 64 * 128], F16)
            nc.sync.dma_start(w_sb[:], wc_d[:])
            w1_sb = singles.tile([128, 48 * 128], F16)
            nc.sync.dma_start(w1_sb[:], w1_d[:])
            bst_sb = singles.tile([8, 128], F16)
            nc.sync.dma_start(bst_sb[:], bst_d[:])
            bni_st = singles.tile([8, 128], F16)
            nc.sync.dma_start(bni_st[:], bni_d[:])
            ones8_sb = singles.tile([8, 8 * b], F16)
            nc.sync.dma_start(ones8_sb[:], ones8_d[:])
            ones4_sb = singles.tile([4, 4 * b], F16)
            nc.sync.dma_start(ones4_sb[:], ones4_d[:])

            h16 = state.tile([128, 4 * b], F16, tag="h16")
            nc.sync.dma_start(h16[:], h0_d[:])

            # Warm-up: have each engine observe the init DMA queues so loop
            # instructions carry at most one embedded cross-engine wait.
            warm_ps = psum.tile([128, 8], F32, tag="warm", bufs=1)
            nc.tensor.matmul(warm_ps[:, 0:8], w_sb[:, 0:128], w_sb[:, 0:8],
                             start=True, stop=True)
            nc.tensor.matmul(warm_ps[:, 0:8], w1_sb[:, 0:128], w1_sb[:, 0:8],
                             start=True, stop=True)
            nc.tensor.matmul(warm_ps[:, 0:1], bst_sb[:, 0:128],
                             ones8_sb[:, 0:1], start=True, stop=True)
            nc.tensor.matmul(warm_ps[:, 0:1], bni_st[:, 0:128],
                             ones8_sb[:, 0:1], start=True, stop=True)

            for t in range(T):
                first = t == 0
                order = TILE_ORDER1 if first else TILE_ORDER
                ntiles = len(order)
                w = w1_sb if first else w_sb

                rz_ps = ps_rz_pool.tile([128, 8 * b], F32, tag="rz")
                ni_ps = ps_ni_pool.tile([128, 8 * b], F32, tag="ni")

                # bias seeds: every start=True for a PSUM bank must precede
                # all accumulating MMs of that bank (first_mm clears the whole
                # bank's has_written bits, not just the addressed region).
                # hn and in share one tile: cols 0:4b hn, 4b:8b in (in region
                # is bias-only at t=0).
                nc.tensor.matmul(rz_ps[:], bst_sb[:], ones8_sb[:],
                                 start=True, stop=False, skip_group_check=True)
                nc.tensor.matmul(ni_ps[:], bni_st[:], ones8_sb[:],
                                 start=True, stop=False, skip_group_check=True)

                for k in range(KT):
                    mv = h16[:, k * b:(k + 1) * b]
                    for ti, (g, c) in enumerate(order):
                        blk = (k * ntiles + ti) * 128
                        if g == "r":
                            ps, off = rz_ps, (c % 2) * b + (c // 2) * 4 * b
                        elif g == "z":
                            ps, off = rz_ps, 2 * b + (c % 2) * b + (c // 2) * 4 * b
                        elif g == "hn":
                            ps, off = ni_ps, c * b
                        else:
                            ps, off = ni_ps, 4 * b + c * b
                        nc.tensor.matmul(
                            ps[:, off:off + b],
                            w[:, blk:blk + 128],
                            mv,
                            start=False,
                            stop=(k == KT - 1),
                            skip_group_check=True,
                        )

                rzs = work.tile([128, 8 * b], F32, tag="rzs")
                rhn = work.tile([128, 4 * b], F32, tag="rhn")
                pre = work.tile([128, 4 * b], F32, tag="pre")
                n_t = work.tile([128, 4 * b], F32, tag="n")
                omz = work.tile([128, 4 * b], F32, tag="omz")
                zh = work.tile([128, 4 * b], F32, tag="zh")
                on = work.tile([128, 4 * b], F32, tag="on")
                h16_new = state.tile([128, 4 * b], F16, tag="h16")

                # rz layout: [r0,r1,z0,z1 | r2,r3,z2,z3]
                def rcol(c):
                    return (c % 2) * b + (c // 2) * 4 * b

                def zcol(c):
                    return 2 * b + (c % 2) * b + (c // 2) * 4 * b

                def chunk_pre(c):
                    cb = slice(c * b, (c + 1) * b)
                    ib = slice(4 * b + c * b, 4 * b + (c + 1) * b)
                    nc.vector.tensor_mul(rhn[:, cb], ni_ps[:, cb],
                                         rzs[:, rcol(c):rcol(c) + b])
                    nc.vector.tensor_add(pre[:, cb], ni_ps[:, ib], rhn[:, cb])
                    nc.scalar.activation(n_t[:, cb], pre[:, cb], tanh)

                def chunk_post(c):
                    cb = slice(c * b, (c + 1) * b)
                    nc.vector.tensor_mul(on[:, cb], omz[:, cb], n_t[:, cb])
                    nc.vector.tensor_add(h16_new[:, cb], on[:, cb], zh[:, cb])

                def half_oz(hf):
                    # off the critical path: run on GpSimd to unload the DVE
                    hb = slice(2 * hf * b, 2 * (hf + 1) * b)
                    zb = slice(zcol(2 * hf), zcol(2 * hf) + 2 * b)
                    nc.gpsimd.tensor_scalar(omz[:, hb], rzs[:, zb], -1.0, 1.0,
                                            MULT, ADD)
                    nc.gpsimd.tensor_mul(zh[:, hb], rzs[:, zb], h16[:, hb])

                nc.scalar.activation(rzs[:, 0:4 * b], rz_ps[:, 0:4 * b], sig)
                chunk_pre(0)
                chunk_pre(1)
                half_oz(0)
                nc.scalar.activation(rzs[:, 4 * b:8 * b], rz_ps[:, 4 * b:8 * b],
                                     sig)
                chunk_post(0)
                chunk_post(1)
                chunk_pre(2)
                chunk_pre(3)
                half_oz(1)
                chunk_post(2)
                chunk_post(3)

                nc.sync.dma_start(out_d[t], h16_new[:])
                h16 = h16_new

    if not nc.is_finalized():
        nc.finalize()
    return nc


def _prep_host(encoder_outputs, W_ih, W_hh, b_ih, b_hh, T, n_cores, b):
    """Shard + lay out host inputs; returns per-core in_maps."""
    W_ih = np.asarray(W_ih, dtype=np.float32)
    W_hh = np.asarray(W_hh, dtype=np.float32)
    b_ih = np.asarray(b_ih, dtype=np.float32)
    b_hh = np.asarray(b_hh, dtype=np.float32)
    enc = np.asarray(encoder_outputs, dtype=np.float32)

    gates = {
        "r": W_ih[:H] + W_hh[:H],
        "z": W_ih[H:2 * H] + W_hh[H:2 * H],
        "hn": W_hh[2 * H:],
        "in": W_ih[2 * H:],
    }
    gates1 = {
        "r": W_hh[:H],
        "z": W_hh[H:2 * H],
        "hn": W_hh[2 * H:],
    }
    b_r = b_ih[:H] + b_hh[:H]
    b_z = b_ih[H:2 * H] + b_hh[H:2 * H]
    b_hn = b_hh[2 * H:]
    b_in = b_ih[2 * H:]

    def layout(gmap, order):
        cols = []
        for k in range(KT):
            for g, c in order:
                Wg = gmap[g]
                blkT = Wg[128 * c:128 * (c + 1), 128 * k:128 * (k + 1)].T
                cols.append(np.ascontiguousarray(blkT))
        return np.concatenate(cols, axis=1).astype(np.float16)

    wc_host = layout(gates, TILE_ORDER)     # (128, 64*128)
    w1_host = layout(gates1, TILE_ORDER1)   # (128, 48*128)

    br4 = b_r.reshape(4, 128)
    bz4 = b_z.reshape(4, 128)
    # rz rows [br0,br1,bz0,bz1, br2,br3,bz2,bz3] matching the psum layout
    bst = np.stack([br4[0], br4[1], bz4[0], bz4[1],
                    br4[2], br4[3], bz4[2], bz4[3]],
                   axis=0).astype(np.float16)  # (8, 128)
    bnist = np.concatenate([b_hn.reshape(4, 128), b_in.reshape(4, 128)],
                           axis=0).astype(np.float16)  # (8, 128)
    ones8 = np.kron(np.eye(8, dtype=np.float16), np.ones((1, b), np.float16))
    ones4 = np.kron(np.eye(4, dtype=np.float16), np.ones((1, b), np.float16))

    h0 = enc[0, :, -1, :]  # (128, 512)
    in_maps = []
    for c in range(n_cores):
        h0c = h0[c * b:(c + 1) * b]  # (b, 512)
        h0t = np.ascontiguousarray(
            h0c.reshape(b, KT, 128).transpose(2, 1, 0).reshape(128, KT * b)
        ).astype(np.float16)
        in_maps.append({
            "wc": wc_host, "w1": w1_host, "bst": bst, "bnist": bnist,
            "ones8": ones8, "ones4": ones4, "h0t": h0t,
        })
    return in_maps


def _gather(results, T, n_cores, b):
    out = np.empty((T, BATCH, H), dtype=np.float32)
    for c in range(n_cores):
        oc = results[c]["outT"].astype(np.float32)  # (T, 128, KT*b)
        out[:, c * b:(c + 1) * b, :] = (
            oc.reshape(T, 128, KT, b).transpose(0, 3, 2, 1).reshape(T, b, H)
        )
    return out


_CACHE = {}


def kernel(encoder_outputs, W_ih, W_hh, b_ih, b_hh, out_len):
    T = int(out_len)
    assert T == T_STEPS, f"built for T={T_STEPS}, got {T}"
    key = (T, N_CORES)
    if key not in _CACHE:
        _CACHE[key] = _build(T, B_LOC)
    nc = _CACHE[key]

    in_maps = _prep_host(encoder_outputs, W_ih, W_hh, b_ih, b_hh,
                         T, N_CORES, B_LOC)
    res = run_bass_kernel_spmd(nc, in_maps, core_ids=list(range(N_CORES)))
    global LAST_RESULTS
    LAST_RESULTS = res
    out = _gather(res.results, T, N_CORES, B_LOC)
    return out.reshape(T * BATCH, 1, H)


LAST_RESULTS = None


# revision 11
# speedup vs baseline: 1.0494x; 1.0484x over previous
"""Trainium2 Bass kernel for the GRU decoder (nn_Decoder_13168369730058).

Math (from the reference):
  h0 = encoder_outputs[0, :, -1, :]                       # (128, 512)
  step 1:   h1 = gru_cell(x=0, h0)
  step t>1: h_t = gru_cell(h_{t-1}, h_{t-1})   (carry is (h_new, h_new))

Because x == h from step 2 on, the two GRU matmuls fuse into one:
  g  = h @ Wc.T          Wc = [Wih_r+Whh_r; Wih_z+Whh_z; Whh_n; Wih_n]  (2048, 512)
  r  = sigmoid(g_r + b_r)        b_r = b_ih_r + b_hh_r
  z  = sigmoid(g_z + b_z)
  n  = tanh(g_in + b_in + r * (g_hn + b_hn))     b_in = b_ih_n, b_hn = b_hh_n
  h' = (1 - z) * n + z * h
Step 1 is the same recurrence with Wc -> W_hh and no in-matmul (g_in = 0).

Distribution: data-parallel over batch, 16 rows per core on 8 cores, weights
replicated; the out_len recurrence is local to each core.

v2 design (from trace analysis of v1: the 64-matmul weight stream runs at the
~27ns/pair PE issue floor = 1.7us, but the serial gate-math tail added 2.2us
of PE idle per step):
  - k-OUTER matmul ordering: phase k uses only h16 chunk k as the moving
    operand, so the next step's stream starts as soon as tail chunk 0 is done.
  - phase k3 is chunk-major (r0,z0,r1,z1,hn0,in0,...) so gate tiles complete
    progressively and the tail starts ~300ns before the stream ends.
  - tail is chunked by k-tile and interleaved across ACT/DVE so h16 chunk 0
    closes the recurrence ~700ns after the stream, with later chunks hidden
    under the next stream's phases.
  - b_hn/b_in biases folded into scalar_tensor_tensor ops; b_r/b_z via one
    cheap bias-seed matmul so one ACT covers r and z sigmoid per half.
  - h carried purely in fp16 (numpy sim: rel err 5.5e-4 vs 2e-2 budget);
    output DMA'd as fp16 and widened to fp32 on the host.
"""

import os
import numpy as np

import concourse.bacc as bacc
import concourse.mybir as mybir
import concourse.tile as tile
from concourse.bass_utils import run_bass_kernel_spmd

H = 512
BATCH = 128
N_CORES = int(os.environ.get("GRU_N_CORES", "8"))
T_STEPS = int(os.environ.get("GRU_T_STEPS", "1024"))
B_LOC = BATCH // N_CORES  # local batch per core (16)
KT = H // 128             # 4 k-tiles

F32 = mybir.dt.float32
F16 = mybir.dt.float16

# full-step stream tile order within each k phase: chunk-major so the tail
# chunks' inputs complete progressively through phase k3
TILE_ORDER = [
    ("r", 0), ("z", 0), ("r", 1), ("z", 1),
    ("hn", 0), ("in", 0), ("hn", 1), ("in", 1),
    ("r", 2), ("z", 2), ("r", 3), ("z", 3),
    ("hn", 2), ("in", 2), ("hn", 3), ("in", 3),
]
# step-1 order (no in gate)
TILE_ORDER1 = [
    ("r", 0), ("z", 0), ("r", 1), ("z", 1), ("hn", 0), ("hn", 1),
    ("r", 2), ("z", 2), ("r", 3), ("z", 3), ("hn", 2), ("hn", 3),
]


def _build(T: int, b: int):
    nc = bacc.Bacc()

    wc_d = nc.dram_tensor("wc", [128, 64 * 128], F16, kind="ExternalInput")
    w1_d = nc.dram_tensor("w1", [128, 48 * 128], F16, kind="ExternalInput")
    # bias stationaries: rz rows [br0,br1,bz0,bz1,br2,br3,bz2,bz3]
    bst01_d = nc.dram_tensor("bst01", [4, 128], F16, kind="ExternalInput")
    bst23_d = nc.dram_tensor("bst23", [4, 128], F16, kind="ExternalInput")
    bni01_d = nc.dram_tensor("bni01", [4, 128], F16, kind="ExternalInput")
    bni23_d = nc.dram_tensor("bni23", [4, 128], F16, kind="ExternalInput")
    ones8_d = nc.dram_tensor("ones8", [8, 8 * b], F16, kind="ExternalInput")
    ones4_d = nc.dram_tensor("ones4", [4, 4 * b], F16, kind="ExternalInput")
    h0_d = nc.dram_tensor("h0t", [128, 4 * b], F16, kind="ExternalInput")
    out_d = nc.dram_tensor("outT", [T, 128, 4 * b], F16, kind="ExternalOutput")

    sig = mybir.ActivationFunctionType.Sigmoid
    tanh = mybir.ActivationFunctionType.Tanh
    ADD = mybir.AluOpType.add
    MULT = mybir.AluOpType.mult

    with tile.TileContext(nc) as tc:
        with (
            tc.tile_pool(name="singles", bufs=1) as singles,
            tc.tile_pool(name="state", bufs=2) as state,
            tc.tile_pool(name="work", bufs=2) as work,
            tc.tile_pool(name="ps_a", bufs=2, space="PSUM") as ps_a,
            tc.tile_pool(name="ps_b", bufs=2, space="PSUM") as ps_b,
            tc.tile_pool(name="ps_c", bufs=2, space="PSUM") as ps_c,
            tc.tile_pool(name="ps_d", bufs=2, space="PSUM") as ps_d,
        ):
            w_sb = singles.tile([128, 64 * 128], F16)
            nc.sync.dma_start(w_sb[:], wc_d[:])
            w1_sb = singles.tile([128, 48 * 128], F16)
            nc.sync.dma_start(w1_sb[:], w1_d[:])
            bst01 = singles.tile([4, 128], F16)
            nc.sync.dma_start(bst01[:], bst01_d[:])
            bst23 = singles.tile([4, 128], F16)
            nc.sync.dma_start(bst23[:], bst23_d[:])
            bni01 = singles.tile([4, 128], F16)
            nc.sync.dma_start(bni01[:], bni01_d[:])
            bni23 = singles.tile([4, 128], F16)
            nc.sync.dma_start(bni23[:], bni23_d[:])
            ones8_sb = singles.tile([8, 8 * b], F16)
            nc.sync.dma_start(ones8_sb[:], ones8_d[:])
            ones4_sb = singles.tile([4, 4 * b], F16)
            nc.sync.dma_start(ones4_sb[:], ones4_d[:])

            h16 = state.tile([128, 4 * b], F16, tag="h16")
            nc.sync.dma_start(h16[:], h0_d[:])



            for t in range(T):
                first = t == 0
                order = TILE_ORDER1 if first else TILE_ORDER
                ntiles = len(order)
                w = w1_sb if first else w_sb

                # four PSUM tiles so the tail's dependencies resolve per
                # tile-group: rz01=[r0,r1,z0,z1], ni01=[hn0,hn1,in0,in1], etc.
                rz01_ps = ps_a.tile([128, 4 * b], F32, tag="rz01")
                rz23_ps = ps_b.tile([128, 4 * b], F32, tag="rz23")
                ni01_ps = ps_c.tile([128, 4 * b], F32, tag="ni01")
                ni23_ps = ps_d.tile([128, 4 * b], F32, tag="ni23")

                # bias seeds: every start=True for a PSUM bank must precede
                # all accumulating MMs of that bank (first_mm clears the whole
                # bank's has_written bits, not just the addressed region). The
                # in regions are bias-only at t=0.
                for ps_t, st in ((rz01_ps, bst01), (rz23_ps, bst23),
                                 (ni01_ps, bni01), (ni23_ps, bni23)):
                    nc.tensor.matmul(ps_t[:], st[:], ones4_sb[:],
                                     start=True, stop=False,
                                     skip_group_check=True)

                for k in range(KT):
                    mv = h16[:, k * b:(k + 1) * b]
                    for ti, (g, c) in enumerate(order):
                        blk = (k * ntiles + ti) * 128
                        lo = c < 2
                        if g == "r":
                            ps, off = (rz01_ps if lo else rz23_ps), (c % 2) * b
                        elif g == "z":
                            ps, off = (rz01_ps if lo else rz23_ps), \
                                2 * b + (c % 2) * b
                        elif g == "hn":
                            ps, off = (ni01_ps if lo else ni23_ps), (c % 2) * b
                        else:
                            ps, off = (ni01_ps if lo else ni23_ps), \
                                2 * b + (c % 2) * b
                        nc.tensor.matmul(
                            ps[:, off:off + b],
                            w[:, blk:blk + 128],
                            mv,
                            start=False,
                            stop=(k == KT - 1),
                            skip_group_check=True,
                        )

                rzs = work.tile([128, 8 * b], F32, tag="rzs")
                rhn = work.tile([128, 4 * b], F32, tag="rhn")
                pre = work.tile([128, 4 * b], F32, tag="pre")
                n_t = work.tile([128, 4 * b], F32, tag="n")
                omz = work.tile([128, 4 * b], F32, tag="omz")
                zh = work.tile([128, 4 * b], F32, tag="zh")
                on = work.tile([128, 4 * b], F32, tag="on")
                h16_new = state.tile([128, 4 * b], F16, tag="h16")

                # tail in halves; rzs layout [r0,r1,z0,z1 | r2,r3,z2,z3]
                def half_pre(hf):
                    hb = slice(2 * hf * b, 2 * (hf + 1) * b)
                    ni = ni01_ps if hf == 0 else ni23_ps
                    rz_lo = slice(4 * hf * b, (4 * hf + 2) * b)
                    nc.vector.tensor_mul(rhn[:, hb], ni[:, 0:2 * b],
                                         rzs[:, rz_lo])
                    nc.vector.tensor_add(pre[:, hb], ni[:, 2 * b:4 * b],
                                         rhn[:, hb])
                    nc.scalar.activation(n_t[:, hb], pre[:, hb], tanh)

                def half_oz(hf):
                    # off the critical path: on GpSimd to unload the DVE
                    hb = slice(2 * hf * b, 2 * (hf + 1) * b)
                    rz_hi = slice((4 * hf + 2) * b, (4 * hf + 4) * b)
                    nc.gpsimd.tensor_scalar(omz[:, hb], rzs[:, rz_hi],
                                            -1.0, 1.0, MULT, ADD)
                    nc.gpsimd.tensor_mul(zh[:, hb], rzs[:, rz_hi], h16[:, hb])

                def half_post(hf):
                    hb = slice(2 * hf * b, 2 * (hf + 1) * b)
                    nc.vector.tensor_mul(on[:, hb], omz[:, hb], n_t[:, hb])
                    nc.vector.tensor_add(h16_new[:, hb], on[:, hb], zh[:, hb])

                nc.scalar.activation(rzs[:, 0:4 * b], rz01_ps[:], sig)
                nc.scalar.activation(rzs[:, 4 * b:8 * b], rz23_ps[:], sig)
                half_pre(0)
                half_oz(0)
                half_pre(1)
                half_oz(1)
                half_post(0)
                half_post(1)

                nc.sync.dma_start(out_d[t], h16_new[:])
                h16 = h16_new

    if not nc.is_finalized():
        nc.finalize()
    return nc


def _prep_host(encoder_outputs, W_ih, W_hh, b_ih, b_hh, T, n_cores, b):
    """Shard + lay out host inputs; returns per-core in_maps."""
    W_ih = np.asarray(W_ih, dtype=np.float32)
    W_hh = np.asarray(W_hh, dtype=np.float32)
    b_ih = np.asarray(b_ih, dtype=np.float32)
    b_hh = np.asarray(b_hh, dtype=np.float32)
    enc = np.asarray(encoder_outputs, dtype=np.float32)

    gates = {
        "r": W_ih[:H] + W_hh[:H],
        "z": W_ih[H:2 * H] + W_hh[H:2 * H],
        "hn": W_hh[2 * H:],
        "in": W_ih[2 * H:],
    }
    gates1 = {
        "r": W_hh[:H],
        "z": W_hh[H:2 * H],
        "hn": W_hh[2 * H:],
    }
    b_r = b_ih[:H] + b_hh[:H]
    b_z = b_ih[H:2 * H] + b_hh[H:2 * H]
    b_hn = b_hh[2 * H:]
    b_in = b_ih[2 * H:]

    def layout(gmap, order):
        cols = []
        for k in range(KT):
            for g, c in order:
                Wg = gmap[g]
                blkT = Wg[128 * c:128 * (c + 1), 128 * k:128 * (k + 1)].T
                cols.append(np.ascontiguousarray(blkT))
        return np.concatenate(cols, axis=1).astype(np.float16)

    wc_host = layout(gates, TILE_ORDER)     # (128, 64*128)
    w1_host = layout(gates1, TILE_ORDER1)   # (128, 48*128)

    br4 = b_r.reshape(4, 128)
    bz4 = b_z.reshape(4, 128)
    # rz rows [br0,br1,bz0,bz1, br2,br3,bz2,bz3] matching the psum layout
    bhn4 = b_hn.reshape(4, 128)
    bin4 = b_in.reshape(4, 128)
    bst01 = np.stack([br4[0], br4[1], bz4[0], bz4[1]]).astype(np.float16)
    bst23 = np.stack([br4[2], br4[3], bz4[2], bz4[3]]).astype(np.float16)
    bni01 = np.stack([bhn4[0], bhn4[1], bin4[0], bin4[1]]).astype(np.float16)
    bni23 = np.stack([bhn4[2], bhn4[3], bin4[2], bin4[3]]).astype(np.float16)
    ones8 = np.kron(np.eye(8, dtype=np.float16), np.ones((1, b), np.float16))
    ones4 = np.kron(np.eye(4, dtype=np.float16), np.ones((1, b), np.float16))

    h0 = enc[0, :, -1, :]  # (128, 512)
    in_maps = []
    for c in range(n_cores):
        h0c = h0[c * b:(c + 1) * b]  # (b, 512)
        h0t = np.ascontiguousarray(
            h0c.reshape(b, KT, 128).transpose(2, 1, 0).reshape(128, KT * b)
        ).astype(np.float16)
        in_maps.append({
            "wc": wc_host, "w1": w1_host, "bst01": bst01, "bst23": bst23,
            "bni01": bni01, "bni23": bni23, "ones8": ones8, "ones4": ones4,
            "h0t": h0t,
        })
    return in_maps


def _gather(results, T, n_cores, b):
    out = np.empty((T, BATCH, H), dtype=np.float32)
    for c in range(n_cores):
        oc = results[c]["outT"].astype(np.float32)  # (T, 128, KT*b)
        out[:, c * b:(c + 1) * b, :] = (
            oc.reshape(T, 128, KT, b).transpose(0, 3, 2, 1).reshape(T, b, H)
        )
    return out


_CACHE = {}


def kernel(encoder_outputs, W_ih, W_hh, b_ih, b_hh, out_len):
    T = int(out_len)
    assert T == T_STEPS, f"built for T={T_STEPS}, got {T}"
    key = (T, N_CORES)
    if key not in _CACHE:
        _CACHE[key] = _build(T, B_LOC)
    nc = _CACHE[key]

    in_maps = _prep_host(encoder_outputs, W_ih, W_hh, b_ih, b_hh,
                         T, N_CORES, B_LOC)
    res = run_bass_kernel_spmd(nc, in_maps, core_ids=list(range(N_CORES)))
    global LAST_RESULTS
    LAST_RESULTS = res
    out = _gather(res.results, T, N_CORES, B_LOC)
    return out.reshape(T * BATCH, 1, H)


LAST_RESULTS = None


# revision 12
# speedup vs baseline: 1.1552x; 1.1008x over previous
"""Trainium2 Bass kernel for the GRU decoder (nn_Decoder_13168369730058).

Math (from the reference):
  h0 = encoder_outputs[0, :, -1, :]                       # (128, 512)
  step 1:   h1 = gru_cell(x=0, h0)
  step t>1: h_t = gru_cell(h_{t-1}, h_{t-1})   (carry is (h_new, h_new))

Because x == h from step 2 on, the two GRU matmuls fuse into one:
  g  = h @ Wc.T          Wc = [Wih_r+Whh_r; Wih_z+Whh_z; Whh_n; Wih_n]  (2048, 512)
  r  = sigmoid(g_r + b_r)        b_r = b_ih_r + b_hh_r
  z  = sigmoid(g_z + b_z)
  n  = tanh(g_in + b_in + r * (g_hn + b_hn))     b_in = b_ih_n, b_hn = b_hh_n
  h' = (1 - z) * n + z * h
Step 1 is the same recurrence with Wc -> W_hh and no in-matmul (g_in = 0).

Distribution: data-parallel over batch, 16 rows per core on 8 cores, weights
replicated; the out_len recurrence is local to each core.
"""

import os
import numpy as np

import concourse.bacc as bacc
import concourse.mybir as mybir
import concourse.tile as tile
from concourse.bass_utils import run_bass_kernel_spmd

H = 512
BATCH = 128
N_CORES = int(os.environ.get("GRU_N_CORES", "8"))
T_STEPS = int(os.environ.get("GRU_T_STEPS", "1024"))
B_LOC = BATCH // N_CORES  # local batch per core (16)
KT = H // 128             # 4 k-tiles

F32 = mybir.dt.float32
F16 = mybir.dt.float16


def _build(T: int, b: int):
    """Build the Bass program: T steps, b batch rows per core."""
    nc = bacc.Bacc()

    wc_d = nc.dram_tensor("wc", [128, 64 * 128], F16, kind="ExternalInput")
    w1_d = nc.dram_tensor("w1", [128, 48 * 128], F16, kind="ExternalInput")
    bst_d = nc.dram_tensor("bst", [16, 128], F16, kind="ExternalInput")
    ones8_d = nc.dram_tensor("ones8", [8, 8 * b], F16, kind="ExternalInput")
    ones4_d = nc.dram_tensor("ones4", [4, 4 * b], F16, kind="ExternalInput")
    h0_d = nc.dram_tensor("h0t", [128, 4 * b], F32, kind="ExternalInput")
    out_d = nc.dram_tensor("outT", [T, 128, 4 * b], F32, kind="ExternalOutput")

    sig = mybir.ActivationFunctionType.Sigmoid
    tanh = mybir.ActivationFunctionType.Tanh

    with tile.TileContext(nc) as tc:
        with (
            tc.tile_pool(name="singles", bufs=1) as singles,
            tc.tile_pool(name="state", bufs=2) as state,
            tc.tile_pool(name="work", bufs=2) as work,
            tc.tile_pool(name="psum", bufs=2, space="PSUM") as psum,
        ):
            w_sb = singles.tile([128, 64 * 128], F16)
            nc.sync.dma_start(w_sb[:], wc_d[:])
            w1_sb = singles.tile([128, 48 * 128], F16)
            nc.sync.dma_start(w1_sb[:], w1_d[:])
            brz_sb = singles.tile([8, 128], F16)
            nc.sync.dma_start(brz_sb[:], bst_d[0:8])
            bhn_sb = singles.tile([4, 128], F16)
            nc.sync.dma_start(bhn_sb[:], bst_d[8:12])
            bin_sb = singles.tile([4, 128], F16)
            nc.sync.dma_start(bin_sb[:], bst_d[12:16])
            ones8_sb = singles.tile([8, 8 * b], F16)
            nc.sync.dma_start(ones8_sb[:], ones8_d[:])
            ones4_sb = singles.tile([4, 4 * b], F16)
            nc.sync.dma_start(ones4_sb[:], ones4_d[:])

            h32 = state.tile([128, 4 * b], F32, tag="h32")
            nc.sync.dma_start(h32[:], h0_d[:])
            h16 = state.tile([128, 4 * b], F16, tag="h16")
            nc.vector.tensor_copy(h16[:], h32[:])

            # Warm-up: hardware allows ONE embedded sync wait per instruction;
            # have each engine observe the init DMA queues here so loop
            # instructions carry a single cross-engine wait.
            warm_ps = psum.tile([128, 8], F32, tag="warm", bufs=1)
            nc.tensor.matmul(warm_ps[:, 0:8], w_sb[:, 0:128], w_sb[:, 0:8],
                             start=True, stop=True)
            nc.tensor.matmul(warm_ps[:, 0:8], w1_sb[:, 0:128], w1_sb[:, 0:8],
                             start=True, stop=True)
            nc.tensor.matmul(warm_ps[:, 0:1], brz_sb[:, 0:128], ones8_sb[:, 0:1],
                             start=True, stop=True)
            nc.tensor.matmul(warm_ps[:, 0:1], bhn_sb[:, 0:128], ones4_sb[:, 0:1],
                             start=True, stop=True)
            nc.tensor.matmul(warm_ps[:, 0:1], bin_sb[:, 0:128], ones4_sb[:, 0:1],
                             start=True, stop=True)

            for t in range(T):
                first = t == 0
                w = w1_sb if first else w_sb

                rz_ps = psum.tile([128, 8 * b], F32, tag="rz")
                hn_ps = psum.tile([128, 4 * b], F32, tag="hn")
                in_ps = psum.tile([128, 4 * b], F32, tag="in")

                # bias seeds (start=True writes bias, sets has_written)
                nc.tensor.matmul(rz_ps[:], brz_sb[:], ones8_sb[:],
                                 start=True, stop=False, skip_group_check=True)
                nc.tensor.matmul(hn_ps[:], bhn_sb[:], ones4_sb[:],
                                 start=True, stop=False, skip_group_check=True)
                nc.tensor.matmul(in_ps[:], bin_sb[:], ones4_sb[:],
                                 start=True, stop=first, skip_group_check=True)

                def mm_block(ps, ps_off, blk0, ntiles):
                    for tt in range(ntiles):
                        for k in range(KT):
                            blk = (blk0 + tt * KT + k) * 128
                            nc.tensor.matmul(
                                ps[:, (ps_off + tt) * b : (ps_off + tt + 1) * b],
                                w[:, blk : blk + 128],
                                h16[:, k * b : (k + 1) * b],
                                start=False,
                                stop=(k == KT - 1),
                                skip_group_check=True,
                            )

                # issue order r, z, hn, in: the rz sigmoid's precise
                # semaphore wait (bias+r+z writers) resolves ~500ns before
                # the stream ends, so sigmoid + r*hn + omz/zh all run in the
                # shadow of the hn/in matmuls; only pre_n onward waits for
                # the in-block.
                mm_block(rz_ps, 0, 0, 4)       # r   (Wc rows 0..511)
                mm_block(rz_ps, 4, 16, 4)      # z   (rows 512..1023)
                mm_block(hn_ps, 0, 32, 4)      # hn  (rows 1024..1535)
                if not first:
                    mm_block(in_ps, 0, 48, 4)  # in  (rows 1536..2047)

                rz_sig = work.tile([128, 8 * b], F32, tag="rz_sig")
                nc.scalar.activation(rz_sig[:], rz_ps[:], sig)
                rT = rz_sig[:, 0 : 4 * b]
                zT = rz_sig[:, 4 * b : 8 * b]

                rhn = work.tile([128, 4 * b], F32, tag="rhn")
                nc.vector.tensor_mul(rhn[:], rT, hn_ps[:])
                omz = work.tile([128, 4 * b], F32, tag="omz")
                nc.vector.tensor_scalar(
                    omz[:], zT, -1.0, 1.0,
                    mybir.AluOpType.mult, mybir.AluOpType.add,
                )
                zh = work.tile([128, 4 * b], F32, tag="zh")
                nc.vector.tensor_mul(zh[:], zT, h32[:])

                pre_n = work.tile([128, 4 * b], F32, tag="pre_n")
                nc.vector.tensor_add(pre_n[:], in_ps[:], rhn[:])
                n_t = work.tile([128, 4 * b], F32, tag="n")
                nc.scalar.activation(n_t[:], pre_n[:], tanh)

                on = work.tile([128, 4 * b], F32, tag="on")
                nc.vector.tensor_mul(on[:], omz[:], n_t[:])
                h16_new = state.tile([128, 4 * b], F16, tag="h16")
                nc.vector.tensor_add(h16_new[:], on[:], zh[:])
                h32_new = state.tile([128, 4 * b], F32, tag="h32")
                nc.vector.tensor_add(h32_new[:], on[:], zh[:])
                nc.sync.dma_start(out_d[t], h32_new[:])
                h16, h32 = h16_new, h32_new

    if not nc.is_finalized():
        nc.finalize()
    return nc


def _prep_host(encoder_outputs, W_ih, W_hh, b_ih, b_hh, T, n_cores, b):
    """Shard + lay out host inputs; returns per-core in_maps."""
    W_ih = np.asarray(W_ih, dtype=np.float32)
    W_hh = np.asarray(W_hh, dtype=np.float32)
    b_ih = np.asarray(b_ih, dtype=np.float32)
    b_hh = np.asarray(b_hh, dtype=np.float32)
    enc = np.asarray(encoder_outputs, dtype=np.float32)

    # combined weights / biases; Wc row blocks ordered [r; z; hn; in]
    Wc = np.concatenate(
        [W_ih[:H] + W_hh[:H], W_ih[H : 2 * H] + W_hh[H : 2 * H],
         W_hh[2 * H :], W_ih[2 * H :]], axis=0,
    )
    W1 = W_hh  # step 1: [r; z; hn], no in-block
    bc_rz = np.concatenate([b_ih[:H] + b_hh[:H], b_ih[H : 2 * H] + b_hh[H : 2 * H]])
    b_hn = b_hh[2 * H :]
    b_in = b_ih[2 * H :]

    def blocks_of(Wm, n_row_tiles):
        WmT = np.ascontiguousarray(Wm.T)  # (512, rows)
        cols = []
        for tt in range(n_row_tiles):
            for k in range(KT):
                cols.append(WmT[128 * k : 128 * (k + 1), 128 * tt : 128 * (tt + 1)])
        return np.concatenate(cols, axis=1).astype(np.float16)

    wc_host = blocks_of(Wc, 16)   # (128, 64*128)
    w1_host = blocks_of(W1, 12)   # (128, 48*128)

    bst = np.concatenate([
        bc_rz.reshape(8, 128), b_hn.reshape(4, 128), b_in.reshape(4, 128),
    ], axis=0).astype(np.float16)  # (16, 128)
    ones8 = np.kron(np.eye(8, dtype=np.float16), np.ones((1, b), np.float16))
    ones4 = np.kron(np.eye(4, dtype=np.float16), np.ones((1, b), np.float16))

    h0 = enc[0, :, -1, :]  # (128, 512)
    in_maps = []
    for c in range(n_cores):
        h0c = h0[c * b : (c + 1) * b]  # (b, 512)
        h0t = np.ascontiguousarray(
            h0c.reshape(b, KT, 128).transpose(2, 1, 0).reshape(128, KT * b)
        ).astype(np.float32)
        in_maps.append({
            "wc": wc_host, "w1": w1_host, "bst": bst,
            "ones8": ones8, "ones4": ones4, "h0t": h0t,
        })
    return in_maps


def _gather(results, T, n_cores, b):
    out = np.empty((T, BATCH, H), dtype=np.float32)
    for c in range(n_cores):
        oc = results[c]["outT"]  # (T, 128, KT*b), free = [k][j]
        out[:, c * b : (c + 1) * b, :] = (
            oc.reshape(T, 128, KT, b).transpose(0, 3, 2, 1).reshape(T, b, H)
        )
    return out


_CACHE = {}


def kernel(encoder_outputs, W_ih, W_hh, b_ih, b_hh, out_len):
    T = int(out_len)
    assert T == T_STEPS, f"built for T={T_STEPS}, got {T}"
    key = (T, N_CORES)
    if key not in _CACHE:
        _CACHE[key] = _build(T, B_LOC)
    nc = _CACHE[key]

    in_maps = _prep_host(encoder_outputs, W_ih, W_hh, b_ih, b_hh,
                         T, N_CORES, B_LOC)
    res = run_bass_kernel_spmd(nc, in_maps, core_ids=list(range(N_CORES)))
    global LAST_RESULTS
    LAST_RESULTS = res
    out = _gather(res.results, T, N_CORES, B_LOC)
    return out.reshape(T * BATCH, 1, H)


LAST_RESULTS = None
